# revision 6
# baseline (speedup 1.0000x reference)
"""Trainium2 Bass kernel for EngramCodebook (vq_codebook).

reference semantics:
    pooled   = hidden_state.mean(axis=0)                  # [2048]
    d[s]     = || seed_bank[s] - pooled ||                # [4096]
    idx      = argmin(d)
    usage_new = usage_frequency + onehot(idx)
    recon    = broadcast(seed_bank[idx], (16384, 2048))

Distribution (8 NeuronCores, column-sharded):
    core i owns columns c_i = [256*i, 256*(i+1)) of hidden_state/seed_bank.
    - local: pool_chunk = mean over rows of hidden[:, c_i]        (PE ones-matmul)
    - local: dpart[s] = ||sb[s,c_i]||^2 - 2*sb[s,c_i].pool_chunk  (ACT square-accum + DVE ttr)
    - AllReduce(add) of dpart [4096]  ->  full d^2 - ||pool||^2 (argmin-equivalent)
    - local: min -> onehot(is_equal) -> matmul-gather winner row chunk -> broadcast
             to recon[:, c_i]; usage/idx computed identically on every core.
"""

import sys

sys.path.insert(0, "/opt/trn_rl_repo")

import numpy as np

N_CORES = 8
N_ROWS = 16384          # hidden_state rows
D = 2048                # state dim
S = 4096                # num seeds
C = D // N_CORES        # columns per core = 256

_CACHE = {}


def _build_program():
    from concourse import bacc, mybir, tile

    f32 = mybir.dt.float32
    i32 = mybir.dt.int32

    nc = bacc.Bacc("TRN2", target_bir_lowering=False, debug=False,
                   num_devices=N_CORES)

    h = nc.dram_tensor("h", [N_ROWS, C], f32, kind="ExternalInput")
    sbk = nc.dram_tensor("sbk", [S, C], f32, kind="ExternalInput")
    uf = nc.dram_tensor("uf", [S], f32, kind="ExternalInput")

    recon = nc.dram_tensor("recon", [N_ROWS, C], f32, kind="ExternalOutput")
    usage = nc.dram_tensor("usage", [S], f32, kind="ExternalOutput")
    sidx = nc.dram_tensor("sidx", [1, 1], f32, kind="ExternalOutput")

    # DRAM views: pack 8 consecutive rows per partition so each DMA is 1 MiB
    # with an 8 KiB contiguous run per partition.
    hv = h.ap().rearrange("(n p e) c -> n p (e c)", n=16, p=128, e=8)
    sbv = sbk.ap().rearrange("(t p e) c -> t p (e c)", t=4, p=128, e=8)
    ufv = uf.ap().rearrange("(q p) -> q p", q=32, p=128)
    usv = usage.ap().rearrange("(q p) -> q p", q=32, p=128)
    rv = recon.ap().rearrange("(n p e) c -> n p (e c)", n=16, p=128, e=8)
    # seed index for element (p, j) of the [128, 32] distance tile:
    #   s = 1024*(j//8) + 8*p + (j%8)
    # and sb_mega[:, 256*j : 256*(j+1)] holds those seeds' column chunk.

    with tile.TileContext(nc) as tc:
        with (
            tc.tile_pool(name="hpool", bufs=3) as hpool,
            tc.tile_pool(name="persist", bufs=1) as persist,
            tc.tile_pool(name="scratch", bufs=2) as scratch,
            tc.tile_pool(name="small", bufs=1) as small,
            tc.tile_pool(name="psum_acc", bufs=1, space="PSUM") as psum_acc,
            tc.tile_pool(name="psum_sm", bufs=3, space="PSUM") as psum_sm,
            tc.tile_pool(name="dram", bufs=1, space="DRAM") as dram,
        ):
            # ---- constants ----
            ones_k = persist.tile([128, 1], f32)        # ones, K-side
            nc.vector.memset(ones_k[:], 1.0)
            ones_m = persist.tile([1, 128], f32)        # ones, M-side (bcast)
            nc.vector.memset(ones_m[:], 1.0)

            iota_pj_i = small.tile([128, 32], i32)
            nc.gpsimd.iota(iota_pj_i[:].rearrange("p (t e) -> p t e", t=4, e=8),
                           pattern=[[1024, 4], [1, 8]], base=0,
                           channel_multiplier=8)
            iota_pj = persist.tile([128, 32], f32)
            nc.vector.tensor_copy(iota_pj[:], iota_pj_i[:])

            iota_nat_i = small.tile([32, 128], i32)
            nc.gpsimd.iota(iota_nat_i[:], pattern=[[1, 128]], base=0,
                           channel_multiplier=128)
            iota_nat = persist.tile([32, 128], f32)
            nc.vector.tensor_copy(iota_nat[:], iota_nat_i[:])

            # ---- load seed bank chunk + usage ----
            sb_mega = persist.tile([128, 32 * C], f32)   # 4 MiB
            for t in range(4):
                nc.sync.dma_start(sb_mega[:, t * 2048:(t + 1) * 2048], sbv[t])
            uf_sb = persist.tile([32, 128], f32)
            nc.sync.dma_start(uf_sb[:], ufv[:])

            # ---- pooled mean: PE ones-matmul accumulation over 16 h tiles ----
            pool_psum = psum_acc.tile([1, 2048], f32)
            for n in range(16):
                h_tile = hpool.tile([128, 2048], f32, name="h_tile")
                nc.sync.dma_start(h_tile[:], hv[n])
                for k in range(4):
                    nc.tensor.matmul(
                        pool_psum[:, k * 512:(k + 1) * 512],
                        ones_k[:],
                        h_tile[:, k * 512:(k + 1) * 512],
                        start=(n == 0), stop=(n == 15),
                    )

            # ---- seed norms: ||sb[s, c_i]||^2 via ACT square + accumulate ----
            sq = persist.tile([128, 32], f32)
            for j in range(32):
                sq_scr = scratch.tile([128, C], f32, name="sq_scr")
                nc.scalar.activation(
                    sq_scr[:], sb_mega[:, j * C:(j + 1) * C],
                    mybir.ActivationFunctionType.Square,
                    accum_out=sq[:, j:j + 1],
                )

            # ---- fold pooled sums [1,2048] -> [1,256], scale by 1/N ----
            pool_flat = small.tile([1, 2048], f32)
            nc.vector.tensor_copy(pool_flat[:], pool_psum[:])
            fold = small.tile([1, 1024], f32)
            nc.vector.tensor_add(fold[:, :1024], pool_flat[:, :1024],
                                 pool_flat[:, 1024:])
            nc.vector.tensor_add(fold[:, :512], fold[:, :512], fold[:, 512:1024])
            nc.vector.tensor_add(fold[:, :256], fold[:, :256], fold[:, 256:512])
            pool_chunk = small.tile([1, C], f32)
            nc.vector.tensor_scalar_mul(pool_chunk[:], fold[:, :256],
                                        1.0 / float(N_ROWS))

            # broadcast pool_chunk across 128 partitions
            pb_psum = psum_sm.tile([128, C], f32, name="pb_psum", tag="ps")
            nc.tensor.matmul(pb_psum[:], ones_m[:], pool_chunk[:])
            poolb = persist.tile([128, C], f32)
            nc.vector.tensor_copy(poolb[:], pb_psum[:])

            # ---- partial NEGATED distances: score[p,j] = 2*sb.pool - sq[p,j]
            # (sign flipped so the winner is the arg-MAX; gpsimd cross-lane
            # reduce only supports add/average/max). DVE does the products,
            # ACT reduces them via copy+accum (tensor_tensor_reduce is broken
            # on HW).
            dot = persist.tile([128, 32], f32)
            for j in range(32):
                mul_scr = scratch.tile([128, C], f32, name="mul_scr")
                nc.vector.tensor_mul(mul_scr[:], sb_mega[:, j * C:(j + 1) * C],
                                     poolb[:])
                acc_scr = scratch.tile([128, C], f32, name="acc_scr")
                nc.scalar.activation(acc_scr[:], mul_scr[:],
                                     mybir.ActivationFunctionType.Copy,
                                     accum_out=dot[:, j:j + 1])
            dmul = persist.tile([128, 32], f32)
            nc.vector.tensor_scalar_mul(dmul[:], dot[:], 2.0)
            dloc = persist.tile([128, 32], f32)
            nc.vector.tensor_sub(dloc[:], dmul[:], sq[:])

            # ---- AllReduce partial distances across the 8 column shards ----
            bounce_in = dram.tile([128, 32], f32)
            bounce_out = dram.tile([128, 32], f32, addr_space="Shared")
            nc.gpsimd.dma_start(bounce_in[:], dloc[:])
            nc.gpsimd.collective_compute(
                "AllReduce",
                mybir.AluOpType.add,
                replica_groups=[list(range(N_CORES))],
                ins=[bounce_in.opt()],
                outs=[bounce_out.opt()],
            )
            dred = persist.tile([128, 32], f32)
            nc.gpsimd.dma_start(dred[:], bounce_out[:])

            # ---- global max of the negated distances ----
            rowmax = small.tile([128, 1], f32)
            nc.vector.tensor_reduce(rowmax[:], dred[:],
                                    axis=mybir.AxisListType.X,
                                    op=mybir.AluOpType.max)
            gmax = small.tile([1, 1], f32)
            nc.gpsimd.tensor_reduce(gmax[:], rowmax[:],
                                    axis=mybir.AxisListType.C,
                                    op=mybir.AluOpType.max)
            gb_psum = psum_sm.tile([128, 1], f32, name="gb_psum", tag="ps")
            nc.tensor.matmul(gb_psum[:], ones_m[:], gmax[:])
            gmax_bc = small.tile([128, 1], f32)
            nc.vector.tensor_copy(gmax_bc[:], gb_psum[:])

            # ---- one-hot of winner (exact f32 equality with the max) ----
            onehot = small.tile([128, 32], f32)
            nc.vector.tensor_scalar(onehot[:], dred[:], gmax_bc[:], None,
                                    op0=mybir.AluOpType.is_equal)

            # ---- seed index = sum(onehot * iota) ----
            masked = small.tile([128, 32], f32)
            nc.vector.tensor_mul(masked[:], onehot[:], iota_pj[:])
            idx_rowsum = small.tile([128, 1], f32)
            nc.vector.tensor_reduce(idx_rowsum[:], masked[:],
                                    axis=mybir.AxisListType.X,
                                    op=mybir.AluOpType.add)
            ix_psum = psum_sm.tile([1, 1], f32, name="ix_psum", tag="ps")
            nc.tensor.matmul(ix_psum[:], ones_k[:], idx_rowsum[:])
            idx_sb = small.tile([1, 1], f32)
            nc.vector.tensor_copy(idx_sb[:], ix_psum[:])
            nc.sync.dma_start(sidx.ap(), idx_sb[:])

            # ---- usage_new = uf + onehot in natural [32,128] layout ----
            ib_psum = psum_sm.tile([32, 1], f32, name="ib_psum", tag="ps")
            nc.tensor.matmul(ib_psum[:], ones_m[:, :32], idx_sb[:])
            idx_bc = small.tile([32, 1], f32)
            nc.vector.tensor_copy(idx_bc[:], ib_psum[:])
            onehot_nat = small.tile([32, 128], f32)
            nc.vector.tensor_scalar(onehot_nat[:], iota_nat[:], idx_bc[:], None,
                                    op0=mybir.AluOpType.is_equal)
            usage_sb = small.tile([32, 128], f32)
            nc.vector.tensor_add(usage_sb[:], onehot_nat[:], uf_sb[:])
            nc.sync.dma_start(usv[:], usage_sb[:])

            # ---- gather winner row chunk: row = onehot^T @ sb_mega ----
            row_psum = psum_sm.tile([1, C], f32, name="row_psum", tag="ps")
            for j in range(32):
                nc.tensor.matmul(row_psum[:], onehot[:, j:j + 1],
                                 sb_mega[:, j * C:(j + 1) * C],
                                 start=(j == 0), stop=(j == 31))
            row_sb = small.tile([1, C], f32)
            nc.vector.tensor_copy(row_sb[:], row_psum[:])

            # broadcast row to 128 partitions, then widen to 8 copies/partition
            rb_psum = psum_sm.tile([128, C], f32, name="rb_psum", tag="ps")
            nc.tensor.matmul(rb_psum[:], ones_m[:], row_sb[:])
            recon_sb = persist.tile([128, 2048], f32)
            nc.vector.tensor_copy(recon_sb[:, :256], rb_psum[:])
            nc.vector.tensor_copy(recon_sb[:, 256:512], recon_sb[:, :256])
            nc.vector.tensor_copy(recon_sb[:, 512:1024], recon_sb[:, :512])
            nc.vector.tensor_copy(recon_sb[:, 1024:2048], recon_sb[:, :1024])

            # ---- write recon chunk: 16 x 1 MiB ----
            for n in range(16):
                nc.sync.dma_start(rv[n], recon_sb[:])

    nc.compile()
    return nc


def _get_program():
    if "nc" not in _CACHE:
        _CACHE["nc"] = _build_program()
    return _CACHE["nc"]


def kernel(hidden_state, seed_bank, usage_frequency):
    from concourse.bass_utils import run_bass_kernel_spmd

    hidden_state = np.asarray(hidden_state, dtype=np.float32)
    seed_bank = np.asarray(seed_bank, dtype=np.float32)
    usage_frequency = np.asarray(usage_frequency, dtype=np.float32)

    nc = _get_program()

    in_maps = []
    for i in range(N_CORES):
        cs = slice(i * C, (i + 1) * C)
        in_maps.append({
            "h": np.ascontiguousarray(hidden_state[:, cs]),
            "sbk": np.ascontiguousarray(seed_bank[:, cs]),
            "uf": usage_frequency,
        })

    res = run_bass_kernel_spmd(nc, in_maps, list(range(N_CORES)))
    results = res.results

    recon = np.concatenate([results[i]["recon"] for i in range(N_CORES)], axis=1)
    usage_new = results[0]["usage"]
    seed_idx = np.int32(np.round(results[0]["sidx"][0, 0]))
    return recon, seed_idx, usage_new


# revision 10
# speedup vs baseline: 1.0784x; 1.0784x over previous
"""Trainium2 Bass kernel for EngramCodebook (vq_codebook).

reference semantics:
    pooled    = hidden_state.mean(axis=0)                 # [2048]
    d[s]      = || seed_bank[s] - pooled ||               # [4096]
    idx       = argmin(d)
    usage_new = usage_frequency + onehot(idx)
    recon     = broadcast(seed_bank[idx], (16384, 2048))

Distribution (8 NeuronCores, column-sharded):
    core i owns columns c_i = [256*i, 256*(i+1)) of hidden_state/seed_bank.
    - local: pool_chunk = mean over rows of hidden[:, c_i]
      (DVE accumulation of row-tiles, partition-summed by PE ones-matmul)
    - local: score[s] = 2*sb[s,c_i].pool_chunk - ||sb[s,c_i]||^2
      (PE bf16 matmuls against a transposed seed bank; ACT square-accum for
      the norms; sign flipped so the winner is the arg-MAX)
    - one 16 KB collective combines partial scores across the 8 shards
    - local: max -> index -> dynamic-slice DMA fetches the winning seed row
      (exact f32) -> broadcast to recon[:, c_i]; usage/idx identical on
      every core.
"""

import os
import sys

sys.path.insert(0, "/opt/trn_rl_repo")

import numpy as np

N_CORES = 8
N_ROWS = 16384          # hidden_state rows
D = 2048                # state dim
S = 4096                # num seeds
C = D // N_CORES        # columns per core = 256

# "AG": AllGather + local sum;  "AR": AllReduce
COLLECTIVE = os.environ.get("VQ_COLLECTIVE", "AG")

_CACHE = {}


def _build_program():
    from concourse import bacc, bass, mybir, tile

    f32 = mybir.dt.float32
    bf16 = mybir.dt.bfloat16
    i32 = mybir.dt.int32

    nc = bacc.Bacc("TRN2", target_bir_lowering=False, debug=False,
                   num_devices=N_CORES)

    h = nc.dram_tensor("h", [N_ROWS, C], f32, kind="ExternalInput")
    sbk = nc.dram_tensor("sbk", [S, C], f32, kind="ExternalInput")
    sbt = nc.dram_tensor("sbt", [C, S], bf16, kind="ExternalInput")
    uf = nc.dram_tensor("uf", [S], f32, kind="ExternalInput")

    recon = nc.dram_tensor("recon", [N_ROWS, C], f32, kind="ExternalOutput")
    usage = nc.dram_tensor("usage", [S], f32, kind="ExternalOutput")
    sidx = nc.dram_tensor("sidx", [1, 1], f32, kind="ExternalOutput")

    # DRAM views. h/recon: pack 8 consecutive rows per partition -> 1 MiB
    # DMAs with 8 KiB contiguous runs. sb: seed-major tiles, 8 seed-blocks
    # per DMA (1 KiB run per (partition, block)).
    hv = h.ap().rearrange("(n p e) c -> n p (e c)", n=16, p=128, e=8)
    sbv = sbk.ap().rearrange("(B b p) c -> B p b c", B=4, b=8, p=128)
    stv = sbt.ap().rearrange("(k p) s -> k p s", k=2, p=128)
    ufv = uf.ap().rearrange("(q p) -> q p", q=32, p=128)
    usv = usage.ap().rearrange("(q p) -> q p", q=32, p=128)
    rv = recon.ap().rearrange("(n p e) c -> n p (e c)", n=16, p=128, e=8)
    # score tile layout: element (p, b) of [128, 32] is seed s = 128*b + p.

    with tile.TileContext(nc) as tc:
        with (
            tc.tile_pool(name="hpool", bufs=3) as hpool,
            tc.tile_pool(name="accp", bufs=2) as accp,
            tc.tile_pool(name="sbp", bufs=2) as sbp,
            tc.tile_pool(name="persist", bufs=1) as persist,
            tc.tile_pool(name="scratch", bufs=2) as scratch,
            tc.tile_pool(name="small", bufs=1) as small,
            tc.tile_pool(name="psum_dot", bufs=1, space="PSUM") as psum_dot,
            tc.tile_pool(name="psum_sm", bufs=3, space="PSUM") as psum_sm,
            tc.tile_pool(name="dram", bufs=1, space="DRAM") as dram,
        ):
            # ---- constants ----
            ones_k = persist.tile([128, 1], f32)
            nc.vector.memset(ones_k[:], 1.0)
            ones_kb = persist.tile([128, 1], bf16)
            nc.vector.memset(ones_kb[:], 1.0)
            ones_m = persist.tile([1, 128], f32)
            nc.vector.memset(ones_m[:], 1.0)
            scale_11 = persist.tile([1, 1], f32)
            nc.vector.memset(scale_11[:], 1.0 / float(N_ROWS))

            iota_pb_i = small.tile([128, 32], i32)
            nc.gpsimd.iota(iota_pb_i[:], pattern=[[128, 32]], base=0,
                           channel_multiplier=1)
            iota_pb = persist.tile([128, 32], f32)
            nc.vector.tensor_copy(iota_pb[:], iota_pb_i[:])

            iota_nat_i = small.tile([32, 128], i32)
            nc.gpsimd.iota(iota_nat_i[:], pattern=[[1, 128]], base=0,
                           channel_multiplier=128)
            iota_nat = persist.tile([32, 128], f32)
            nc.vector.tensor_copy(iota_nat[:], iota_nat_i[:])

            # ---- loads ----
            uf_sb = persist.tile([32, 128], f32)
            nc.sync.dma_start(uf_sb[:], ufv[:])

            sbt_sb = persist.tile([128, 2 * S], bf16)   # 2 MiB, persists
            for k in range(2):
                nc.sync.dma_start(sbt_sb[:, k * S:(k + 1) * S], stv[k])

            # seed norms ||sb[s, c_i]||^2, streamed; (p, b) layout
            sq = persist.tile([128, 32], f32)
            for B in range(4):
                sb_tile = sbp.tile([128, 2048], f32, name="sb_tile")
                nc.sync.dma_start(
                    sb_tile[:].rearrange("p (b c) -> p b c", b=8), sbv[B])
                for b in range(8):
                    sq_scr = scratch.tile([128, C], f32, name="sq_scr")
                    nc.scalar.activation(sq_scr[:], sb_tile[:, b * C:(b + 1) * C],
                                         mybir.ActivationFunctionType.Square,
                                         accum_out=sq[:, 8 * B + b:8 * B + b + 1])

            # hidden tiles: DVE running accumulation (ping-pong)
            prev = None
            for n in range(16):
                h_tile = hpool.tile([128, 2048], f32, name="h_tile")
                nc.sync.dma_start(h_tile[:], hv[n])
                if prev is None:
                    prev = h_tile
                else:
                    acc = accp.tile([128, 2048], f32, name="acc")
                    nc.vector.tensor_add(acc[:], prev[:], h_tile[:])
                    prev = acc

            # ---- pooled sums -> pool_chunk (scaled) transposed to bf16 ----
            acc_bf = persist.tile([128, 2048], bf16)
            nc.vector.tensor_copy(acc_bf[:], prev[:])
            pool_psum = psum_sm.tile([1, C], f32, name="pool_psum", tag="ps")
            for e in range(8):
                nc.tensor.matmul(pool_psum[:], ones_kb[:],
                                 acc_bf[:, e * C:(e + 1) * C],
                                 start=(e == 0), stop=(e == 7))
            pool_sb = small.tile([1, C], f32)
            nc.vector.tensor_copy(pool_sb[:], pool_psum[:])
            # transpose [1,256] -> [256(2x128), 1] with the 1/N scale fused
            ptp_psum = psum_sm.tile([128, 2], f32, name="ptp_psum", tag="ps")
            for k in range(2):
                nc.tensor.matmul(ptp_psum[:, k:k + 1],
                                 pool_sb[:, k * 128:(k + 1) * 128],
                                 scale_11[:])
            poolT = persist.tile([128, 2], bf16)
            nc.vector.tensor_copy(poolT[:], ptp_psum[:])

            # ---- dots on PE: dot[p,b] = sum_c sbt[c, 128b+p] * poolT[c] ----
            dot_psum = psum_dot.tile([128, 32], f32)
            for b in range(32):
                for k in range(2):
                    nc.tensor.matmul(
                        dot_psum[:, b:b + 1],
                        sbt_sb[:, k * S + b * 128:k * S + (b + 1) * 128],
                        poolT[:, k:k + 1],
                        start=(k == 0), stop=(k == 1),
                    )
            dmul = small.tile([128, 32], f32)
            nc.vector.tensor_scalar_mul(dmul[:], dot_psum[:], 2.0)
            dloc = persist.tile([128, 32], f32)
            nc.vector.tensor_sub(dloc[:], dmul[:], sq[:])

            # ---- combine partial scores across the 8 column shards ----
            bounce_in = dram.tile([128, 32], f32)
            nc.gpsimd.dma_start(bounce_in[:], dloc[:])
            dred = persist.tile([128, 32], f32)
            if COLLECTIVE == "AG":
                bounce_out = dram.tile([N_CORES * 128, 32], f32,
                                       addr_space="Shared")
                nc.gpsimd.collective_compute(
                    "AllGather",
                    mybir.AluOpType.bypass,
                    replica_groups=[list(range(N_CORES))],
                    ins=[bounce_in.opt()],
                    outs=[bounce_out.opt()],
                )
                gath = persist.tile([128, 256], f32)
                for g in range(N_CORES):
                    nc.gpsimd.dma_start(gath[:, g * 32:(g + 1) * 32],
                                        bounce_out[g * 128:(g + 1) * 128, :])
                t4 = small.tile([128, 128], f32)
                nc.vector.tensor_add(t4[:], gath[:, :128], gath[:, 128:])
                t2 = small.tile([128, 64], f32)
                nc.vector.tensor_add(t2[:], t4[:, :64], t4[:, 64:])
                nc.vector.tensor_add(dred[:], t2[:, :32], t2[:, 32:])
            else:
                bounce_out = dram.tile([128, 32], f32, addr_space="Shared")
                nc.gpsimd.collective_compute(
                    "AllReduce",
                    mybir.AluOpType.add,
                    replica_groups=[list(range(N_CORES))],
                    ins=[bounce_in.opt()],
                    outs=[bounce_out.opt()],
                )
                nc.gpsimd.dma_start(dred[:], bounce_out[:])

            # ---- global max of scores ----
            rowmax = small.tile([128, 1], f32)
            nc.vector.tensor_reduce(rowmax[:], dred[:],
                                    axis=mybir.AxisListType.X,
                                    op=mybir.AluOpType.max)
            gmax = small.tile([1, 1], f32)
            nc.gpsimd.tensor_reduce(gmax[:], rowmax[:],
                                    axis=mybir.AxisListType.C,
                                    op=mybir.AluOpType.max)
            gb_psum = psum_sm.tile([128, 1], f32, name="gb_psum", tag="ps")
            nc.tensor.matmul(gb_psum[:], ones_m[:], gmax[:])
            gmax_bc = small.tile([128, 1], f32)
            nc.vector.tensor_copy(gmax_bc[:], gb_psum[:])

            # ---- one-hot of winner -> seed index ----
            onehot = small.tile([128, 32], f32)
            nc.vector.tensor_scalar(onehot[:], dred[:], gmax_bc[:], None,
                                    op0=mybir.AluOpType.is_equal)
            masked = small.tile([128, 32], f32)
            nc.vector.tensor_mul(masked[:], onehot[:], iota_pb[:])
            idx_rowsum = small.tile([128, 1], f32)
            nc.vector.tensor_reduce(idx_rowsum[:], masked[:],
                                    axis=mybir.AxisListType.X,
                                    op=mybir.AluOpType.add)
            ix_psum = psum_sm.tile([1, 1], f32, name="ix_psum", tag="ps")
            nc.tensor.matmul(ix_psum[:], ones_k[:], idx_rowsum[:])
            idx_sb = small.tile([1, 1], f32)
            nc.vector.tensor_copy(idx_sb[:], ix_psum[:])
            nc.sync.dma_start(sidx.ap(), idx_sb[:])
            idx_i32 = small.tile([1, 1], i32)
            nc.vector.tensor_copy(idx_i32[:], idx_sb[:])

            # ---- usage_new = uf + onehot in natural [32,128] layout ----
            ib_psum = psum_sm.tile([32, 1], f32, name="ib_psum", tag="ps")
            nc.tensor.matmul(ib_psum[:], ones_m[:, :32], idx_sb[:])
            idx_bc = small.tile([32, 1], f32)
            nc.vector.tensor_copy(idx_bc[:], ib_psum[:])
            onehot_nat = small.tile([32, 128], f32)
            nc.vector.tensor_scalar(onehot_nat[:], iota_nat[:], idx_bc[:], None,
                                    op0=mybir.AluOpType.is_equal)
            usage_sb = small.tile([32, 128], f32)
            nc.vector.tensor_add(usage_sb[:], onehot_nat[:], uf_sb[:])
            nc.sync.dma_start(usv[:], usage_sb[:])

            # ---- fetch winning seed row (exact f32) via dynamic-slice DMA ----
            row_sb = small.tile([1, C], f32)
            with tc.tile_critical():
                with (
                    nc.gpsimd.register("rowidx") as ridx,
                    nc.semaphore("row_sem") as rsem,
                ):
                    nc.gpsimd.reg_load(ridx, idx_i32[:1, :1])
                    off = nc.gpsimd.snap(ridx)
                    nc.gpsimd.dma_start(
                        row_sb[:], sbk.ap()[bass.ds(off, 1), :]
                    ).then_inc(rsem, 16)
                    nc.gpsimd.wait_ge(rsem, 16)

            # broadcast row to 128 partitions, widen to 8 copies/partition
            rb_psum = psum_sm.tile([128, C], f32, name="rb_psum", tag="ps")
            nc.tensor.matmul(rb_psum[:], ones_m[:], row_sb[:])
            recon_sb = persist.tile([128, 2048], f32)
            nc.vector.tensor_copy(recon_sb[:, :256], rb_psum[:])
            nc.vector.tensor_copy(recon_sb[:, 256:512], recon_sb[:, :256])
            nc.vector.tensor_copy(recon_sb[:, 512:1024], recon_sb[:, :512])
            nc.vector.tensor_copy(recon_sb[:, 1024:2048], recon_sb[:, :1024])

            # ---- write recon chunk: 16 x 1 MiB ----
            for n in range(16):
                nc.sync.dma_start(rv[n], recon_sb[:])

    nc.compile()
    return nc


def _get_program():
    if "nc" not in _CACHE:
        _CACHE["nc"] = _build_program()
    return _CACHE["nc"]


def _shard_inputs(hidden_state, seed_bank, usage_frequency):
    import ml_dtypes

    in_maps = []
    for i in range(N_CORES):
        cs = slice(i * C, (i + 1) * C)
        sb_chunk = np.ascontiguousarray(seed_bank[:, cs])
        in_maps.append({
            "h": np.ascontiguousarray(hidden_state[:, cs]),
            "sbk": sb_chunk,
            "sbt": np.ascontiguousarray(sb_chunk.T).astype(ml_dtypes.bfloat16),
            "uf": usage_frequency,
        })
    return in_maps


def kernel(hidden_state, seed_bank, usage_frequency):
    from concourse.bass_utils import run_bass_kernel_spmd

    hidden_state = np.asarray(hidden_state, dtype=np.float32)
    seed_bank = np.asarray(seed_bank, dtype=np.float32)
    usage_frequency = np.asarray(usage_frequency, dtype=np.float32)

    nc = _get_program()
    in_maps = _shard_inputs(hidden_state, seed_bank, usage_frequency)

    res = run_bass_kernel_spmd(nc, in_maps, list(range(N_CORES)))
    results = res.results

    recon = np.concatenate([results[i]["recon"] for i in range(N_CORES)], axis=1)
    usage_new = results[0]["usage"]
    seed_idx = np.int32(np.round(results[0]["sidx"][0, 0]))
    return recon, seed_idx, usage_new


# revision 11
# speedup vs baseline: 1.1252x; 1.0434x over previous
"""Trainium2 Bass kernel for EngramCodebook (vq_codebook).

reference semantics:
    pooled    = hidden_state.mean(axis=0)                 # [2048]
    d[s]      = || seed_bank[s] - pooled ||               # [4096]
    idx       = argmin(d)
    usage_new = usage_frequency + onehot(idx)
    recon     = broadcast(seed_bank[idx], (16384, 2048))

Distribution (8 NeuronCores, column-sharded):
    core i owns columns c_i = [256*i, 256*(i+1)) of hidden_state/seed_bank.
    - local: pool_chunk = mean over rows of hidden[:, c_i]
      (bf16 cast-DMA + DVE running sum, partition-summed by PE ones-matmul)
    - local: score[s] = 2*sb[s,c_i].pool_chunk - ||sb[s,c_i]||^2
      (PE bf16 matmuls against a transposed+permuted seed bank; ACT
      square-accum for the norms; sign flipped -> winner is the arg-MAX)
    - one 16 KB AllReduce combines partial scores across the 8 shards
    - local: max -> index -> dynamic-slice DMA fetches the winning seed row
      (exact f32) -> broadcast to recon[:, c_i]; usage/idx identical on
      every core.

Score-tile layout: element (p, j) of the [128, 32] tiles is seed
    s = 1024*(j//8) + 8*p + (j%8)
which makes every seed-bank DMA fully contiguous; the host permutes the
columns of the transposed bf16 seed bank to match (idx map below).
"""

import os
import sys

sys.path.insert(0, "/opt/trn_rl_repo")

import numpy as np

N_CORES = 8
N_ROWS = 16384          # hidden_state rows
D = 2048                # state dim
S = 4096                # num seeds
C = D // N_CORES        # columns per core = 256

COLLECTIVE = os.environ.get("VQ_COLLECTIVE", "AR")

_CACHE = {}


def _seed_perm():
    # column position (128*b + p) of the permuted sbt holds seed
    # s = 1024*(b//8) + 8*p + (b%8)
    b = np.arange(32)[:, None]
    p = np.arange(128)[None, :]
    return (1024 * (b // 8) + 8 * p + (b % 8)).reshape(-1)


def _build_program():
    from concourse import bacc, bass, mybir, tile
    from concourse import bass_isa

    f32 = mybir.dt.float32
    bf16 = mybir.dt.bfloat16
    i32 = mybir.dt.int32

    nc = bacc.Bacc("TRN2", target_bir_lowering=False, debug=False,
                   num_devices=N_CORES)

    h = nc.dram_tensor("h", [N_ROWS, C], f32, kind="ExternalInput")
    sbk = nc.dram_tensor("sbk", [S, C], f32, kind="ExternalInput")
    sbt = nc.dram_tensor("sbt", [C, S], bf16, kind="ExternalInput")
    uf = nc.dram_tensor("uf", [S], f32, kind="ExternalInput")

    recon = nc.dram_tensor("recon", [N_ROWS, C], f32, kind="ExternalOutput")
    usage = nc.dram_tensor("usage", [S], f32, kind="ExternalOutput")
    sidx = nc.dram_tensor("sidx", [1, 1], f32, kind="ExternalOutput")

    # DRAM views; every DMA has >=1 KiB contiguous runs per partition.
    hv = h.ap().rearrange("(n p e) c -> n p (e c)", n=16, p=128, e=8)
    sbv = sbk.ap().rearrange("(B p e) c -> B p (e c)", B=4, p=128, e=8)
    stv = sbt.ap().rearrange("(k p) s -> k p s", k=2, p=128)
    ufv = uf.ap().rearrange("(q p) -> q p", q=32, p=128)
    usv = usage.ap().rearrange("(q p) -> q p", q=32, p=128)
    rv = recon.ap().rearrange("(n p e) c -> n p (e c)", n=16, p=128, e=8)

    with tile.TileContext(nc) as tc:
        with (
            tc.tile_pool(name="hpool", bufs=3) as hpool,
            tc.tile_pool(name="accp", bufs=2) as accp,
            tc.tile_pool(name="sbp", bufs=2) as sbp,
            tc.tile_pool(name="persist", bufs=1) as persist,
            tc.tile_pool(name="scratch", bufs=2) as scratch,
            tc.tile_pool(name="small", bufs=1) as small,
            tc.tile_pool(name="psum_dot", bufs=1, space="PSUM") as psum_dot,
            tc.tile_pool(name="psum_sm", bufs=3, space="PSUM") as psum_sm,
            tc.tile_pool(name="dram", bufs=1, space="DRAM") as dram,
        ):
            # ---- constants ----
            ones_kb = persist.tile([128, 1], bf16)
            nc.vector.memset(ones_kb[:], 1.0)
            ones_m = persist.tile([1, 128], f32)
            nc.vector.memset(ones_m[:], 1.0)
            scale_11 = persist.tile([1, 1], f32)
            nc.vector.memset(scale_11[:], 1.0 / float(N_ROWS))

            iota_pj_i = small.tile([128, 32], i32)
            nc.gpsimd.iota(iota_pj_i[:].rearrange("p (t e) -> p t e", t=4, e=8),
                           pattern=[[1024, 4], [1, 8]], base=0,
                           channel_multiplier=8)
            iota_pj = persist.tile([128, 32], f32)
            nc.vector.tensor_copy(iota_pj[:], iota_pj_i[:])

            iota_nat_i = small.tile([32, 128], i32)
            nc.gpsimd.iota(iota_nat_i[:], pattern=[[1, 128]], base=0,
                           channel_multiplier=128)
            iota_nat = persist.tile([32, 128], f32)
            nc.vector.tensor_copy(iota_nat[:], iota_nat_i[:])

            # ---- loads (sync queue: sbt, sb, uf; gpsimd queue: h w/ cast) --
            sbt_sb = persist.tile([128, 2 * S], bf16)   # 2 MiB, persists
            for k in range(2):
                nc.sync.dma_start(sbt_sb[:, k * S:(k + 1) * S], stv[k])

            # seed norms ||sb[s, c_i]||^2, streamed; weird (p, j) layout
            sq = persist.tile([128, 32], f32)
            for B in range(4):
                sb_tile = sbp.tile([128, 2048], f32, name="sb_tile")
                nc.sync.dma_start(sb_tile[:], sbv[B])
                for e in range(8):
                    sq_scr = scratch.tile([128, C], f32, name="sq_scr")
                    nc.scalar.activation(sq_scr[:], sb_tile[:, e * C:(e + 1) * C],
                                         mybir.ActivationFunctionType.Square,
                                         accum_out=sq[:, 8 * B + e:8 * B + e + 1])

            uf_sb = persist.tile([32, 128], f32)
            nc.sync.dma_start(uf_sb[:], ufv[:])

            # hidden tiles: cast to bf16 during DMA, DVE running sum
            prev = None
            for n in range(16):
                h_tile = hpool.tile([128, 2048], bf16, name="h_tile")
                nc.gpsimd.dma_start(h_tile[:], hv[n])
                if prev is None:
                    prev = h_tile
                else:
                    acc = accp.tile([128, 2048], bf16, name="acc")
                    nc.vector.tensor_add(acc[:], prev[:], h_tile[:])
                    prev = acc

            # ---- pooled sums -> pool_chunk (scaled), transposed to bf16 ----
            pool_psum = psum_sm.tile([1, C], f32, name="pool_psum", tag="ps")
            for e in range(8):
                nc.tensor.matmul(pool_psum[:], ones_kb[:],
                                 prev[:, e * C:(e + 1) * C],
                                 start=(e == 0), stop=(e == 7))
            pool_sb = small.tile([1, C], f32)
            nc.vector.tensor_copy(pool_sb[:], pool_psum[:])
            # transpose [1,256] -> [256(2x128), 1] with the 1/N scale fused
            ptp_psum = psum_sm.tile([128, 2], f32, name="ptp_psum", tag="ps")
            for k in range(2):
                nc.tensor.matmul(ptp_psum[:, k:k + 1],
                                 pool_sb[:, k * 128:(k + 1) * 128],
                                 scale_11[:])
            poolT = persist.tile([128, 2], bf16)
            nc.vector.tensor_copy(poolT[:], ptp_psum[:])

            # ---- dots on PE: dot[p,j] = sum_c sbt_perm[c, 128j+p]*poolT[c] --
            dot_psum = psum_dot.tile([128, 32], f32)
            for b in range(32):
                for k in range(2):
                    nc.tensor.matmul(
                        dot_psum[:, b:b + 1],
                        sbt_sb[:, k * S + b * 128:k * S + (b + 1) * 128],
                        poolT[:, k:k + 1],
                        start=(k == 0), stop=(k == 1),
                    )
            dmul = small.tile([128, 32], f32)
            nc.vector.tensor_scalar_mul(dmul[:], dot_psum[:], 2.0)
            dloc = persist.tile([128, 32], f32)
            nc.vector.tensor_sub(dloc[:], dmul[:], sq[:])

            # ---- combine partial scores across the 8 column shards ----
            bounce_in = dram.tile([128, 32], f32)
            nc.gpsimd.dma_start(bounce_in[:], dloc[:])
            dred = persist.tile([128, 32], f32)
            if COLLECTIVE == "AG":
                bounce_out = dram.tile([N_CORES * 128, 32], f32,
                                       addr_space="Shared")
                nc.gpsimd.collective_compute(
                    "AllGather",
                    mybir.AluOpType.bypass,
                    replica_groups=[list(range(N_CORES))],
                    ins=[bounce_in.opt()],
                    outs=[bounce_out.opt()],
                )
                gath = persist.tile([128, 256], f32)
                for g in range(N_CORES):
                    nc.gpsimd.dma_start(gath[:, g * 32:(g + 1) * 32],
                                        bounce_out[g * 128:(g + 1) * 128, :])
                t4 = small.tile([128, 128], f32)
                nc.vector.tensor_add(t4[:], gath[:, :128], gath[:, 128:])
                t2 = small.tile([128, 64], f32)
                nc.vector.tensor_add(t2[:], t4[:, :64], t4[:, 64:])
                nc.vector.tensor_add(dred[:], t2[:, :32], t2[:, 32:])
            else:
                bounce_out = dram.tile([128, 32], f32, addr_space="Shared")
                nc.gpsimd.collective_compute(
                    "AllReduce",
                    mybir.AluOpType.add,
                    replica_groups=[list(range(N_CORES))],
                    ins=[bounce_in.opt()],
                    outs=[bounce_out.opt()],
                )
                nc.gpsimd.dma_start(dred[:], bounce_out[:])

            # ---- global max of scores (value broadcast to all partitions) --
            rowmax = small.tile([128, 1], f32)
            nc.vector.tensor_reduce(rowmax[:], dred[:],
                                    axis=mybir.AxisListType.X,
                                    op=mybir.AluOpType.max)
            gmax_bc = small.tile([128, 1], f32)
            nc.gpsimd.partition_all_reduce(gmax_bc[:], rowmax[:], channels=128,
                                           reduce_op=bass_isa.ReduceOp.max)

            # ---- one-hot of winner -> seed index (on all partitions) ----
            onehot = small.tile([128, 32], f32)
            nc.vector.tensor_scalar(onehot[:], dred[:], gmax_bc[:], None,
                                    op0=mybir.AluOpType.is_equal)
            masked = small.tile([128, 32], f32)
            nc.vector.tensor_mul(masked[:], onehot[:], iota_pj[:])
            idx_rowsum = small.tile([128, 1], f32)
            nc.vector.tensor_reduce(idx_rowsum[:], masked[:],
                                    axis=mybir.AxisListType.X,
                                    op=mybir.AluOpType.add)
            idx_all = small.tile([128, 1], f32)
            nc.gpsimd.partition_all_reduce(idx_all[:], idx_rowsum[:],
                                           channels=128,
                                           reduce_op=bass_isa.ReduceOp.add)
            nc.sync.dma_start(sidx.ap(), idx_all[:1, :])
            idx_i32 = small.tile([1, 1], i32)
            nc.vector.tensor_copy(idx_i32[:], idx_all[:1, :])

            # ---- usage_new = uf + onehot in natural [32,128] layout ----
            onehot_nat = small.tile([32, 128], f32)
            nc.vector.tensor_scalar(onehot_nat[:], iota_nat[:],
                                    idx_all[:32, :], None,
                                    op0=mybir.AluOpType.is_equal)
            usage_sb = small.tile([32, 128], f32)
            nc.vector.tensor_add(usage_sb[:], onehot_nat[:], uf_sb[:])
            nc.sync.dma_start(usv[:], usage_sb[:])

            # ---- fetch winning seed row (exact f32) via dynamic-slice DMA --
            row_sb = small.tile([1, C], f32)
            with tc.tile_critical():
                with (
                    nc.gpsimd.register("rowidx") as ridx,
                    nc.semaphore("row_sem") as rsem,
                ):
                    nc.gpsimd.reg_load(ridx, idx_i32[:1, :1])
                    off = nc.gpsimd.snap(ridx)
                    nc.gpsimd.dma_start(
                        row_sb[:], sbk.ap()[bass.ds(off, 1), :]
                    ).then_inc(rsem, 16)
                    nc.gpsimd.wait_ge(rsem, 16)

            # broadcast row to 128 partitions, widen to 8 copies/partition
            rb_psum = psum_sm.tile([128, C], f32, name="rb_psum", tag="ps")
            nc.tensor.matmul(rb_psum[:], ones_m[:], row_sb[:])
            recon_sb = persist.tile([128, 2048], f32)
            nc.vector.tensor_copy(recon_sb[:, :256], rb_psum[:])
            nc.vector.tensor_copy(recon_sb[:, 256:512], recon_sb[:, :256])
            nc.vector.tensor_copy(recon_sb[:, 512:1024], recon_sb[:, :512])
            nc.vector.tensor_copy(recon_sb[:, 1024:2048], recon_sb[:, :1024])

            # ---- write recon chunk: 16 x 1 MiB ----
            for n in range(16):
                nc.sync.dma_start(rv[n], recon_sb[:])

    nc.compile()
    return nc


def _get_program():
    if "nc" not in _CACHE:
        _CACHE["nc"] = _build_program()
    return _CACHE["nc"]


def _shard_inputs(hidden_state, seed_bank, usage_frequency):
    import ml_dtypes

    perm = _seed_perm()
    in_maps = []
    for i in range(N_CORES):
        cs = slice(i * C, (i + 1) * C)
        sb_chunk = np.ascontiguousarray(seed_bank[:, cs])
        sbt_perm = np.ascontiguousarray(
            sb_chunk.T[:, perm]).astype(ml_dtypes.bfloat16)
        in_maps.append({
            "h": np.ascontiguousarray(hidden_state[:, cs]),
            "sbk": sb_chunk,
            "sbt": sbt_perm,
            "uf": usage_frequency,
        })
    return in_maps


def kernel(hidden_state, seed_bank, usage_frequency):
    from concourse.bass_utils import run_bass_kernel_spmd

    hidden_state = np.asarray(hidden_state, dtype=np.float32)
    seed_bank = np.asarray(seed_bank, dtype=np.float32)
    usage_frequency = np.asarray(usage_frequency, dtype=np.float32)

    nc = _get_program()
    in_maps = _shard_inputs(hidden_state, seed_bank, usage_frequency)

    res = run_bass_kernel_spmd(nc, in_maps, list(range(N_CORES)))
    results = res.results

    recon = np.concatenate([results[i]["recon"] for i in range(N_CORES)], axis=1)
    usage_new = results[0]["usage"]
    seed_idx = np.int32(np.round(results[0]["sidx"][0, 0]))
    return recon, seed_idx, usage_new


# revision 12
# speedup vs baseline: 1.2342x; 1.0969x over previous
"""Trainium2 Bass kernel for EngramCodebook (vq_codebook).

reference semantics:
    pooled    = hidden_state.mean(axis=0)                 # [2048]
    d[s]      = || seed_bank[s] - pooled ||               # [4096]
    idx       = argmin(d)
    usage_new = usage_frequency + onehot(idx)
    recon     = broadcast(seed_bank[idx], (16384, 2048))

Distribution (8 NeuronCores, column-sharded):
    core i owns columns c_i = [256*i, 256*(i+1)) of hidden_state/seed_bank.
    - local: pool_chunk = mean over rows of hidden[:, c_i]
      (dual-queue loads, bf16 DVE running sum, PE ones-matmul partition sum)
    - local: score[s] = 2*sb[s,c_i].pool_chunk - ||sb[s,c_i]||^2
      (PE bf16 matmuls against a transposed+permuted seed bank; ACT
      square-accum for the norms; sign flipped -> winner is the arg-MAX)
    - one 16 KB AllReduce combines partial scores across the 8 shards
    - local: max -> index -> dynamic-slice DMA fetches the winning seed row
      (exact f32) -> broadcast to recon[:, c_i]; usage/idx identical on
      every core.

Score-tile layout: element (p, j) of the [128, 32] tiles is seed
    s = 1024*(j//8) + 8*p + (j%8)
which makes every seed-bank DMA fully contiguous; the host permutes the
columns of the transposed bf16 seed bank to match (idx map below).
"""

import os
import sys

sys.path.insert(0, "/opt/trn_rl_repo")

import numpy as np

N_CORES = 8
N_ROWS = 16384          # hidden_state rows
D = 2048                # state dim
S = 4096                # num seeds
C = D // N_CORES        # columns per core = 256

COLLECTIVE = os.environ.get("VQ_COLLECTIVE", "AR")

_CACHE = {}


def _seed_perm():
    # column position (128*b + p) of the permuted sbt holds seed
    # s = 1024*(b//8) + 8*p + (b%8)
    b = np.arange(32)[:, None]
    p = np.arange(128)[None, :]
    return (1024 * (b // 8) + 8 * p + (b % 8)).reshape(-1)


def _build_program():
    from concourse import bacc, bass, mybir, tile

    f32 = mybir.dt.float32
    bf16 = mybir.dt.bfloat16
    i32 = mybir.dt.int32

    nc = bacc.Bacc("TRN2", target_bir_lowering=False, debug=False,
                   num_devices=N_CORES)

    h = nc.dram_tensor("h", [N_ROWS, C], f32, kind="ExternalInput")
    sbk = nc.dram_tensor("sbk", [S, C], f32, kind="ExternalInput")
    sbt = nc.dram_tensor("sbt", [C, S], bf16, kind="ExternalInput")
    uf = nc.dram_tensor("uf", [S], f32, kind="ExternalInput")

    recon = nc.dram_tensor("recon", [N_ROWS, C], f32, kind="ExternalOutput")
    usage = nc.dram_tensor("usage", [S], f32, kind="ExternalOutput")
    sidx = nc.dram_tensor("sidx", [1, 1], f32, kind="ExternalOutput")

    # DRAM views; every DMA has >=1 KiB contiguous runs per partition.
    hv = h.ap().rearrange("(n p e) c -> n p (e c)", n=16, p=128, e=8)
    sbv = sbk.ap().rearrange("(B p e) c -> B p (e c)", B=4, p=128, e=8)
    stv = sbt.ap().rearrange("(k p) s -> k p s", k=2, p=128)
    ufv = uf.ap().rearrange("(q p) -> q p", q=32, p=128)
    usv = usage.ap().rearrange("(q p) -> q p", q=32, p=128)
    rv = recon.ap().rearrange("(n p e) c -> n p (e c)", n=16, p=128, e=8)

    with tile.TileContext(nc) as tc:
        with (
            tc.tile_pool(name="hpool", bufs=4) as hpool,
            tc.tile_pool(name="accp", bufs=2) as accp,
            tc.tile_pool(name="sbp", bufs=2) as sbp,
            tc.tile_pool(name="persist", bufs=1) as persist,
            tc.tile_pool(name="scratch", bufs=2) as scratch,
            tc.tile_pool(name="small", bufs=1) as small,
            tc.tile_pool(name="psum_dot", bufs=1, space="PSUM") as psum_dot,
            tc.tile_pool(name="psum_sm", bufs=3, space="PSUM") as psum_sm,
            tc.tile_pool(name="dram", bufs=1, space="DRAM") as dram,
        ):
            # ---- kick off the big loads first ----
            # transposed bf16 seed bank (sync queue), persists for the dots
            sbt_sb = persist.tile([128, 2 * S], bf16)
            for k in range(2):
                nc.sync.dma_start(sbt_sb[:, k * S:(k + 1) * S], stv[k])

            # f32 seed bank tiles (sync queue), streamed for the norms
            sb_tiles = []
            for B in range(4):
                sb_tile = sbp.tile([128, 2048], f32, name="sb_tile")
                nc.sync.dma_start(sb_tile[:], sbv[B])
                sb_tiles.append(sb_tile)

            # hidden tiles: even tiles on sync (f32 + DVE cast), odd tiles on
            # gpsimd (cast-to-bf16 during DMA) so both DMA queues stay busy
            h_bf = []
            for n in range(16):
                if n % 2 == 0:
                    h_f = hpool.tile([128, 2048], f32, name="h_f")
                    nc.sync.dma_start(h_f[:], hv[n])
                    h_b = hpool.tile([128, 2048], bf16, name="h_b")
                    nc.vector.tensor_copy(h_b[:], h_f[:])
                else:
                    h_b = hpool.tile([128, 2048], bf16, name="h_b")
                    nc.gpsimd.dma_start(h_b[:], hv[n])
                h_bf.append(h_b)

            uf_sb = persist.tile([32, 128], f32)
            nc.sync.dma_start(uf_sb[:], ufv[:])

            # ---- constants (scheduled into load-phase idle time) ----
            ones_kb = persist.tile([128, 1], bf16)
            nc.vector.memset(ones_kb[:], 1.0)
            ones_m = persist.tile([1, 128], f32)
            nc.vector.memset(ones_m[:], 1.0)
            scale_11 = persist.tile([1, 1], f32)
            nc.vector.memset(scale_11[:], 1.0 / float(N_ROWS))

            iota_pj_i = small.tile([128, 32], i32)
            nc.gpsimd.iota(iota_pj_i[:].rearrange("p (t e) -> p t e", t=4, e=8),
                           pattern=[[1024, 4], [1, 8]], base=0,
                           channel_multiplier=8)
            iota_pj = persist.tile([128, 32], f32)
            nc.vector.tensor_copy(iota_pj[:], iota_pj_i[:])

            iota_nat_i = small.tile([32, 128], i32)
            nc.gpsimd.iota(iota_nat_i[:], pattern=[[1, 128]], base=0,
                           channel_multiplier=128)
            iota_nat = persist.tile([32, 128], f32)
            nc.vector.tensor_copy(iota_nat[:], iota_nat_i[:])

            # identity matrix for PE transposes of [128,1] vectors
            iota_id_i = small.tile([128, 128], i32)
            nc.gpsimd.iota(iota_id_i[:], pattern=[[1, 128]], base=0,
                           channel_multiplier=-1)
            idf = small.tile([128, 128], f32)
            nc.vector.tensor_copy(idf[:], iota_id_i[:])
            ident = persist.tile([128, 128], f32)
            nc.vector.tensor_scalar(ident[:], idf[:], 0.0, None,
                                    op0=mybir.AluOpType.is_equal)

            # ---- seed norms ||sb[s, c_i]||^2 (ACT, hidden under loads) ----
            sq = persist.tile([128, 32], f32)
            for B in range(4):
                for e in range(8):
                    sq_scr = scratch.tile([128, C], f32, name="sq_scr")
                    nc.scalar.activation(sq_scr[:],
                                         sb_tiles[B][:, e * C:(e + 1) * C],
                                         mybir.ActivationFunctionType.Square,
                                         accum_out=sq[:, 8 * B + e:8 * B + e + 1])

            # ---- bf16 running sum of hidden tiles (DVE, DMA-paced) ----
            prev = h_bf[0]
            for n in range(1, 16):
                acc = accp.tile([128, 2048], bf16, name="acc")
                nc.vector.tensor_add(acc[:], prev[:], h_bf[n][:])
                prev = acc

            # ---- pooled sums -> pool_chunk (scaled), transposed to bf16 ----
            pool_psum = psum_sm.tile([1, C], f32, name="pool_psum", tag="ps")
            for e in range(8):
                nc.tensor.matmul(pool_psum[:], ones_kb[:],
                                 prev[:, e * C:(e + 1) * C],
                                 start=(e == 0), stop=(e == 7))
            pool_sb = small.tile([1, C], f32)
            nc.vector.tensor_copy(pool_sb[:], pool_psum[:])
            # transpose [1,256] -> [256(2x128), 1] with the 1/N scale fused
            ptp_psum = psum_sm.tile([128, 2], f32, name="ptp_psum", tag="ps")
            for k in range(2):
                nc.tensor.matmul(ptp_psum[:, k:k + 1],
                                 pool_sb[:, k * 128:(k + 1) * 128],
                                 scale_11[:])
            poolT = persist.tile([128, 2], bf16)
            nc.vector.tensor_copy(poolT[:], ptp_psum[:])

            # ---- dots on PE: dot[p,j] = sum_c sbt_perm[c, 128j+p]*poolT[c] --
            dot_psum = psum_dot.tile([128, 32], f32)
            for b in range(32):
                for k in range(2):
                    nc.tensor.matmul(
                        dot_psum[:, b:b + 1],
                        sbt_sb[:, k * S + b * 128:k * S + (b + 1) * 128],
                        poolT[:, k:k + 1],
                        start=(k == 0), stop=(k == 1),
                    )
            dmul = small.tile([128, 32], f32)
            nc.vector.tensor_scalar_mul(dmul[:], dot_psum[:], 2.0)
            dloc = persist.tile([128, 32], f32)
            nc.vector.tensor_sub(dloc[:], dmul[:], sq[:])

            # ---- combine partial scores across the 8 column shards ----
            bounce_in = dram.tile([128, 32], f32)
            nc.gpsimd.dma_start(bounce_in[:], dloc[:])
            dred = persist.tile([128, 32], f32)
            bounce_out = dram.tile([128, 32], f32, addr_space="Shared")
            nc.gpsimd.collective_compute(
                "AllReduce",
                mybir.AluOpType.add,
                replica_groups=[list(range(N_CORES))],
                ins=[bounce_in.opt()],
                outs=[bounce_out.opt()],
            )
            nc.sync.dma_start(dred[:], bounce_out[:])

            # ---- global max of scores (PE transpose + DVE reduce) ----
            rowmax = small.tile([128, 1], f32)
            nc.vector.tensor_reduce(rowmax[:], dred[:],
                                    axis=mybir.AxisListType.X,
                                    op=mybir.AluOpType.max)
            rmt_psum = psum_sm.tile([1, 128], f32, name="rmt_psum", tag="ps")
            nc.tensor.transpose(rmt_psum[:], rowmax[:], ident[:])
            rmt_sb = small.tile([1, 128], f32)
            nc.vector.tensor_copy(rmt_sb[:], rmt_psum[:])
            gmax = small.tile([1, 1], f32)
            nc.vector.tensor_reduce(gmax[:], rmt_sb[:],
                                    axis=mybir.AxisListType.X,
                                    op=mybir.AluOpType.max)
            gb_psum = psum_sm.tile([128, 1], f32, name="gb_psum", tag="ps")
            nc.tensor.matmul(gb_psum[:], ones_m[:], gmax[:])
            gmax_bc = small.tile([128, 1], f32)
            nc.vector.tensor_copy(gmax_bc[:], gb_psum[:])

            # ---- one-hot of winner -> seed index ----
            onehot = small.tile([128, 32], f32)
            nc.vector.tensor_scalar(onehot[:], dred[:], gmax_bc[:], None,
                                    op0=mybir.AluOpType.is_equal)
            masked = small.tile([128, 32], f32)
            nc.vector.tensor_mul(masked[:], onehot[:], iota_pj[:])
            idx_rowsum = small.tile([128, 1], f32)
            nc.vector.tensor_reduce(idx_rowsum[:], masked[:],
                                    axis=mybir.AxisListType.X,
                                    op=mybir.AluOpType.add)
            ixt_psum = psum_sm.tile([1, 128], f32, name="ixt_psum", tag="ps")
            nc.tensor.transpose(ixt_psum[:], idx_rowsum[:], ident[:])
            ixt_sb = small.tile([1, 128], f32)
            nc.vector.tensor_copy(ixt_sb[:], ixt_psum[:])
            idx_sb = small.tile([1, 1], f32)
            nc.vector.tensor_reduce(idx_sb[:], ixt_sb[:],
                                    axis=mybir.AxisListType.X,
                                    op=mybir.AluOpType.add)
            nc.sync.dma_start(sidx.ap(), idx_sb[:])
            idx_i32 = small.tile([1, 1], i32)
            nc.vector.tensor_copy(idx_i32[:], idx_sb[:])

            # ---- usage_new = uf + onehot in natural [32,128] layout ----
            ib_psum = psum_sm.tile([32, 1], f32, name="ib_psum", tag="ps")
            nc.tensor.matmul(ib_psum[:], ones_m[:, :32], idx_sb[:])
            idx_bc = small.tile([32, 1], f32)
            nc.vector.tensor_copy(idx_bc[:], ib_psum[:])
            onehot_nat = small.tile([32, 128], f32)
            nc.vector.tensor_scalar(onehot_nat[:], iota_nat[:], idx_bc[:], None,
                                    op0=mybir.AluOpType.is_equal)
            usage_sb = small.tile([32, 128], f32)
            nc.vector.tensor_add(usage_sb[:], onehot_nat[:], uf_sb[:])
            nc.sync.dma_start(usv[:], usage_sb[:])

            # ---- fetch winning seed row (exact f32) via dynamic-slice DMA --
            row_sb = small.tile([1, C], f32)
            with tc.tile_critical():
                with (
                    nc.gpsimd.register("rowidx") as ridx,
                    nc.semaphore("row_sem") as rsem,
                ):
                    nc.gpsimd.reg_load(ridx, idx_i32[:1, :1])
                    off = nc.gpsimd.snap(ridx)
                    nc.gpsimd.dma_start(
                        row_sb[:], sbk.ap()[bass.ds(off, 1), :]
                    ).then_inc(rsem, 16)
                    nc.gpsimd.wait_ge(rsem, 16)

            # broadcast row to 128 partitions, widen to 8 copies/partition
            rb_psum = psum_sm.tile([128, C], f32, name="rb_psum", tag="ps")
            nc.tensor.matmul(rb_psum[:], ones_m[:], row_sb[:])
            recon_sb = persist.tile([128, 2048], f32)
            nc.vector.tensor_copy(recon_sb[:, :256], rb_psum[:])
            nc.vector.tensor_copy(recon_sb[:, 256:512], recon_sb[:, :256])
            nc.vector.tensor_copy(recon_sb[:, 512:1024], recon_sb[:, :512])
            nc.vector.tensor_copy(recon_sb[:, 1024:2048], recon_sb[:, :1024])

            # ---- write recon chunk: 16 x 1 MiB ----
            for n in range(16):
                nc.sync.dma_start(rv[n], recon_sb[:])

    nc.compile()
    return nc


def _get_program():
    if "nc" not in _CACHE:
        _CACHE["nc"] = _build_program()
    return _CACHE["nc"]


def _shard_inputs(hidden_state, seed_bank, usage_frequency):
    import ml_dtypes

    perm = _seed_perm()
    in_maps = []
    for i in range(N_CORES):
        cs = slice(i * C, (i + 1) * C)
        sb_chunk = np.ascontiguousarray(seed_bank[:, cs])
        sbt_perm = np.ascontiguousarray(
            sb_chunk.T[:, perm]).astype(ml_dtypes.bfloat16)
        in_maps.append({
            "h": np.ascontiguousarray(hidden_state[:, cs]),
            "sbk": sb_chunk,
            "sbt": sbt_perm,
            "uf": usage_frequency,
        })
    return in_maps


def kernel(hidden_state, seed_bank, usage_frequency):
    from concourse.bass_utils import run_bass_kernel_spmd

    hidden_state = np.asarray(hidden_state, dtype=np.float32)
    seed_bank = np.asarray(seed_bank, dtype=np.float32)
    usage_frequency = np.asarray(usage_frequency, dtype=np.float32)

    nc = _get_program()
    in_maps = _shard_inputs(hidden_state, seed_bank, usage_frequency)

    res = run_bass_kernel_spmd(nc, in_maps, list(range(N_CORES)))
    results = res.results

    recon = np.concatenate([results[i]["recon"] for i in range(N_CORES)], axis=1)
    usage_new = results[0]["usage"]
    seed_idx = np.int32(np.round(results[0]["sidx"][0, 0]))
    return recon, seed_idx, usage_new
